# Initial kernel scaffold
#
"""Trainium2 Bass kernel for nn_Bert2DLinear (12-layer linear-attention BERT2D encoder).

Sharding: data-parallel over batch (B=8) across the 8 NeuronCores; one batch
element (S=2048 tokens) per core. Weights/tables replicated per core.

Per-core internal layout:
  h_bf  : residual stream, feature-major [128 part = f%128, 6 = f//128, 2048 tokens] bf16
  K_bf  : token-major      [128 part = s%128, 16 = s//128, 768 feats] bf16
  V_bf  : token-major      [128, 16, 6, 129] bf16 (col 128 of each chunk = 1.0 for ksum)
  Q/att : feature-major per 512-token group
All matmuls bf16 (PSUM accumulation fp32) except small fp32r helpers.
"""
import os
import sys

sys.path.insert(0, '/opt/trn_rl_repo')

import numpy as np
import ml_dtypes

import concourse.bass as bass
import concourse.tile as tile
from concourse import bacc, mybir
from concourse import bass_utils
from concourse.bass import ds, ts

F32 = mybir.dt.float32
F32R = mybir.dt.float32r
BF16 = mybir.dt.bfloat16
I16 = mybir.dt.int16
AF = mybir.ActivationFunctionType

B, S, D, L, H, DH = 8, 2048, 768, 12, 12, 64
LN_EPS = 1e-12
DEN_EPS = 1e-6
NC = 6            # feature chunks of 128
NT = 16           # token tiles of 128
NG = 4            # token groups of 512
N_LAYERS = int(os.environ.get("KERNEL_LAYERS", str(L)))

_CACHED = {}


def _emit(nc, tc, ap):
    sing = tc.tile_pool(name="sing", bufs=1).__enter__()
    wpool = tc.tile_pool(name="wpool", bufs=1).__enter__()
    rows = tc.tile_pool(name="rows", bufs=8).__enter__()
    xp = tc.tile_pool(name="xp", bufs=2).__enter__()
    x2p = tc.tile_pool(name="x2p", bufs=2).__enter__()
    qp = tc.tile_pool(name="qp", bufs=4).__enter__()
    attp = tc.tile_pool(name="attp", bufs=2).__enter__()
    drp = tc.tile_pool(name="drp", bufs=2).__enter__()
    brow = tc.tile_pool(name="brow", bufs=2).__enter__()
    psum = tc.tile_pool(name="psum", bufs=8, space="PSUM").__enter__()

    def mmtile(shape=(128, 512), dtype=F32):
        return psum.tile(list(shape), dtype, tag="mm")

    # ---- persistent SBUF ----
    h_bf = sing.tile([128, NC, S], BF16, tag="h")
    K_bf = sing.tile([128, NT, D], BF16, tag="K")
    V_bf = sing.tile([128, NT, NC, 129], BF16, tag="V")
    kv2 = sing.tile([128, NC, 128], BF16, tag="kv2")
    DSb = sing.tile([128, NC, 128], BF16, tag="DS")
    ksum = sing.tile([128, NC], F32, tag="ksum")
    idx = sing.tile([128, 6, 128], I16, tag="idx")
    maskt = sing.tile([128, NT], F32, tag="mask")
    ident_bf = sing.tile([128, 128], BF16, tag="idbf")
    ident_f = sing.tile([128, 128], F32, tag="idf")
    ones64 = sing.tile([128, 64], BF16, tag="o64")
    ones_r128_bf = sing.tile([1, 128], BF16, tag="o1x128b")
    ones_r128_f = sing.tile([1, 128], F32, tag="o1x128f")
    eps_r128_f = sing.tile([1, 128], F32, tag="e1x128f")
    ones_r512_f = sing.tile([1, 512], F32, tag="o1x512f")
    ones_c_bf = sing.tile([128, 1], BF16, tag="ocbf")
    ones_c_f = sing.tile([128, 1], F32, tag="ocf")
    bqo = sing.tile([128, 2, L * NC], F32, tag="bqo")

    # ---- load constants ----
    nc.sync.dma_start(out=idx, in_=ap["tokidx"].rearrange("t p s -> p t s"))
    nc.sync.dma_start(out=maskt, in_=ap["masktok"])
    nc.sync.dma_start(out=ident_bf, in_=ap["ident_bf"])
    nc.sync.dma_start(out=ident_f, in_=ap["ident_f"])
    nc.sync.dma_start(out=bqo[:, 0, :], in_=ap["bq_t"])
    nc.sync.dma_start(out=bqo[:, 1, :], in_=ap["bo_t"])
    nc.vector.memset(ones64, 1.0)
    nc.vector.memset(ones_r128_bf, 1.0)
    nc.vector.memset(ones_r128_f, 1.0)
    nc.vector.memset(eps_r128_f, DEN_EPS)
    nc.vector.memset(ones_r512_f, 1.0)
    nc.vector.memset(ones_c_bf, 1.0)
    nc.vector.memset(ones_c_f, 1.0)
    nc.vector.memset(V_bf[:, :, :, 128:129], 1.0)
    nc.vector.memset(kv2, 0.0)
    nc.vector.memset(DSb, 0.0)

    tabs = [("word_emb", 0), ("x_emb", 1), ("x_emb", 2),
            ("y_emb", 3), ("y_emb", 4), ("page_emb", 5)]

    # ================= embedding: gather + sum + LN -> token-major in K_bf =========
    for g in range(NG):
        psE = [mmtile((128, 384)) for _ in range(8)]  # (c_local, half)
        for tnum, (tname, tslot) in enumerate(tabs):
            tmp = xp.tile([128, 4, D], F32, tag="x")
            nc.gpsimd.dma_gather(
                out_ap=tmp,
                in_ap=ap[tname],
                idxs_ap=idx[:, tslot, ts(g, 32)],
                num_idxs=512,
                num_idxs_reg=512,
                elem_size=D,
            )
            for c in range(4):
                for hh in range(2):
                    nc.tensor.matmul(
                        psE[2 * c + hh],
                        ident_f.bitcast(F32R),
                        tmp[:, c, ds(384 * hh, 384)].bitcast(F32R),
                        start=(tnum == 0), stop=(tnum == 5),
                    )
        # layernorm per token (free-dim over 768) and write token-major into K_bf
        for c in range(4):
            tc_abs = g * 4 + c  # global token tile
            st = rows.tile([128, 2, 6], F32, tag="bnst")
            mv = rows.tile([128, 4], F32, tag="bnmv")
            nc.vector.bn_stats(out=st[:, 0, :], in_=psE[2 * c + 0][:, 0:384])
            nc.vector.bn_stats(out=st[:, 1, :], in_=psE[2 * c + 1][:, 0:384])
            nc.vector.bn_aggr(out=mv[:, 0:2], in_=st)
            # rstd = 1/sqrt(var + eps)
            nc.scalar.activation(out=mv[:, 2:3], in_=mv[:, 1:2], func=AF.Sqrt,
                                 bias=LN_EPS)
            nc.vector.reciprocal(out=mv[:, 3:4], in_=mv[:, 2:3])
            for hh in range(2):
                nc.vector.tensor_scalar(
                    out=K_bf[:, tc_abs, ds(384 * hh, 384)],
                    in0=psE[2 * c + hh],
                    scalar1=mv[:, 0:1], scalar2=mv[:, 3:4],
                    op0=mybir.AluOpType.subtract, op1=mybir.AluOpType.mult,
                )
    # transpose token-major K_bf -> feature-major h_bf
    for fc in range(NC):
        for g in range(NG):
            pT = mmtile((128, 512), BF16)
            for j in range(4):
                nc.tensor.transpose(
                    pT[:, ts(j, 128)],
                    K_bf[:, g * 4 + j, ts(fc, 128)],
                    ident_bf,
                )
            nc.scalar.activation(out=h_bf[:, fc, ts(g, 512)], in_=pT, func=AF.Copy)

    # ===================== layers =====================
    for l in range(N_LAYERS):
        wq = wpool.tile([128, NC, D], BF16, tag="wq")
        wk = wpool.tile([128, NC, D], BF16, tag="wk")
        wv = wpool.tile([128, NC, D], BF16, tag="wv")
        wo = wpool.tile([128, NC, D], BF16, tag="wo")
        nc.sync.dma_start(out=wk, in_=ap["wk"][l])
        nc.sync.dma_start(out=wv, in_=ap["wv"][l])
        nc.sync.dma_start(out=wq, in_=ap["wq"][l])
        nc.sync.dma_start(out=wo, in_=ap["wo"][l])
        bk = brow.tile([1, D], BF16, tag="bk")
        bv = brow.tile([1, D], BF16, tag="bv")
        nc.sync.dma_start(out=bk, in_=ap["bk_r"][l:l + 1])
        nc.sync.dma_start(out=bv, in_=ap["bv_r"][l:l + 1])

        # ---- stage A1: K/V projections (token-major) ----
        for t in range(NT):
            for (w, brow_t, out_tok, is_k) in ((wk, bk, K_bf, True), (wv, bv, V_bf, False)):
                pa = mmtile()
                pb = mmtile((128, 256))
                for kc in range(NC):
                    lhs = h_bf[:, kc, ts(t, 128)]
                    nc.tensor.matmul(pa, lhs, w[:, kc, 0:512],
                                     start=(kc == 0), stop=False)
                    nc.tensor.matmul(pb, lhs, w[:, kc, 512:768],
                                     start=(kc == 0), stop=False)
                nc.tensor.matmul(pa, ones_r128_bf, brow_t[0:1, 0:512],
                                 start=False, stop=True)
                nc.tensor.matmul(pb, ones_r128_bf, brow_t[0:1, 512:768],
                                 start=False, stop=True)
                if is_k:
                    nc.scalar.activation(out=K_bf[:, t, 0:512], in_=pa, func=AF.Silu)
                    nc.scalar.activation(out=K_bf[:, t, 512:768], in_=pb, func=AF.Silu)
                    nc.vector.tensor_scalar_mul(K_bf[:, t, :], K_bf[:, t, :],
                                                maskt[:, t:t + 1])
                else:
                    nc.vector.tensor_copy(
                        out=V_bf[:, t, 0:4, 0:128],
                        in_=pa.rearrange("p (c e) -> p c e", e=128))
                    nc.vector.tensor_copy(
                        out=V_bf[:, t, 4:6, 0:128],
                        in_=pb.rearrange("p (c e) -> p c e", e=128))

        # ---- stage A2: kv + ksum, build DS ----
        for c in range(NC):
            pkv = mmtile((128, 129))
            for t in range(NT):
                nc.tensor.matmul(pkv, K_bf[:, t, ts(c, 128)], V_bf[:, t, c, :],
                                 start=(t == 0), stop=(t == NT - 1))
            nc.scalar.activation(out=kv2[0:64, c, 0:64], in_=pkv[0:64, 0:64],
                                 func=AF.Copy)
            nc.scalar.activation(out=kv2[64:128, c, 64:128], in_=pkv[64:128, 64:128],
                                 func=AF.Copy)
            nc.scalar.activation(out=ksum[:, c:c + 1], in_=pkv[:, 128:129],
                                 func=AF.Copy)
            nc.vector.tensor_scalar_mul(DSb[0:64, c, 0:64], ones64[0:64, :],
                                        ksum[0:64, c:c + 1])
            nc.vector.tensor_scalar_mul(DSb[64:128, c, 64:128], ones64[64:128, :],
                                        ksum[64:128, c:c + 1])

        # ---- stages B-D per 512-token group ----
        for g in range(NG):
            gs = ts(g, 512)
            att = attp.tile([128, NC, 512], BF16, tag="att")
            xg = xp.tile([128, NC, 512], F32, tag="x")
            x2g = x2p.tile([128, NC, 512], BF16, tag="x2")
            for c in range(NC):
                # Q projection chunk c (feature-major)
                pq = mmtile()
                for kc in range(NC):
                    nc.tensor.matmul(pq, wq[:, kc, ts(c, 128)], h_bf[:, kc, gs],
                                     start=(kc == 0), stop=(kc == NC - 1))
                qt = qp.tile([128, 512], BF16, tag="q")
                nc.scalar.activation(out=qt, in_=pq, func=AF.Silu,
                                     bias=bqo[:, 0, l * NC + c:l * NC + c + 1])
                # num / den
                pn = mmtile()
                nc.tensor.matmul(pn, kv2[:, c, :], qt, start=True, stop=True)
                pd = mmtile()
                nc.tensor.matmul(pd, DSb[:, c, :], qt, start=True, stop=False)
                nc.tensor.matmul(pd, eps_r128_f.bitcast(F32R),
                                 ones_r512_f.bitcast(F32R), start=False, stop=True)
                dr = drp.tile([128, 512], F32, tag="dr")
                nc.vector.reciprocal(out=dr, in_=pd)
                nc.vector.tensor_mul(att[:, c, :], pn, dr)
            # O projection + residual
            for mc in range(NC):
                po = mmtile()
                for kc in range(NC):
                    nc.tensor.matmul(po, wo[:, kc, ts(mc, 128)], att[:, kc, :],
                                     start=(kc == 0), stop=False)
                nc.tensor.matmul(po, ident_bf, h_bf[:, mc, gs],
                                 start=False, stop=True)
                nc.scalar.activation(out=xg[:, mc, :], in_=po, func=AF.Identity,
                                     bias=bqo[:, 1, l * NC + mc:l * NC + mc + 1])
                nc.vector.tensor_mul(x2g[:, mc, :], xg[:, mc, :], xg[:, mc, :])
            # stats
            ps1 = mmtile((1, 512))
            ps2 = mmtile((1, 512))
            for kc in range(NC):
                nc.tensor.matmul(ps1, ones_c_f.bitcast(F32R),
                                 xg[:, kc, :].bitcast(F32R),
                                 start=(kc == 0), stop=(kc == NC - 1))
                nc.tensor.matmul(ps2, ones_c_bf, x2g[:, kc, :],
                                 start=(kc == 0), stop=(kc == NC - 1))
            r_mu = rows.tile([1, 512], F32, tag="row")
            r_e2 = rows.tile([1, 512], F32, tag="row")
            r_tmp = rows.tile([1, 512], F32, tag="row")
            r_rs = rows.tile([1, 512], F32, tag="row")
            nc.scalar.activation(out=r_mu, in_=ps1, func=AF.Copy, scale=1.0 / D)
            nc.scalar.activation(out=r_e2, in_=ps2, func=AF.Copy, scale=1.0 / D)
            nc.vector.tensor_mul(r_tmp, r_mu, r_mu)
            nc.vector.tensor_sub(r_e2, r_e2, r_tmp)
            nc.scalar.activation(out=r_tmp, in_=r_e2, func=AF.Sqrt, bias=LN_EPS)
            nc.vector.reciprocal(out=r_rs, in_=r_tmp)
            nc.vector.tensor_mul(r_mu, r_mu, r_rs)
            pb1 = mmtile()
            pb2 = mmtile()
            nc.tensor.matmul(pb1, ones_r128_f.bitcast(F32R), r_rs.bitcast(F32R),
                             start=True, stop=True)
            nc.tensor.matmul(pb2, ones_r128_f.bitcast(F32R), r_mu.bitcast(F32R),
                             start=True, stop=True)
            for mc in range(NC):
                nc.vector.tensor_mul(xg[:, mc, :], xg[:, mc, :], pb1)
                nc.vector.tensor_sub(h_bf[:, mc, gs], xg[:, mc, :], pb2)

    # ===================== output: transpose to token-major, DMA out ============
    out_d = ap["out"].rearrange("(g c p) m -> p g c m", p=128, c=4)
    for g in range(NG):
        og = xp.tile([128, 4, D], F32, tag="x")
        for j in range(4):
            pa = mmtile((128, 512), BF16)
            pb = mmtile((128, 256), BF16)
            for fc in range(NC):
                dst = pa[:, ts(fc, 128)] if fc < 4 else pb[:, ts(fc - 4, 128)]
                nc.tensor.transpose(dst, h_bf[:, fc, ts(g * 4 + j, 128)], ident_bf)
            nc.scalar.activation(out=og[:, j, 0:512], in_=pa, func=AF.Copy)
            nc.scalar.activation(out=og[:, j, 512:768], in_=pb, func=AF.Copy)
        nc.sync.dma_start(out=out_d[:, g], in_=og)

    for p in (sing, wpool, rows, xp, x2p, qp, attp, drp, brow, psum):
        p.__exit__(None, None, None)


def _build():
    key = N_LAYERS
    if key in _CACHED:
        return _CACHED[key]
    nc = bacc.Bacc("TRN2", target_bir_lowering=False, debug=False, num_devices=8)
    ap = {}

    def din(name, shape, dt):
        ap[name] = nc.dram_tensor(name, list(shape), dt, kind="ExternalInput").ap()

    din("tokidx", (6, 128, 128), I16)
    din("masktok", (128, NT), F32)
    din("ident_bf", (128, 128), BF16)
    din("ident_f", (128, 128), F32)
    din("bq_t", (128, L * NC), F32)
    din("bo_t", (128, L * NC), F32)
    din("bk_r", (L, D), BF16)
    din("bv_r", (L, D), BF16)
    din("word_emb", (21128, D), F32)
    din("x_emb", (1000, D), F32)
    din("y_emb", (1000, D), F32)
    din("page_emb", (2, D), F32)
    for n in ("wq", "wk", "wv", "wo"):
        din(n, (L, 128, NC, D), BF16)
    ap["out"] = nc.dram_tensor("out", [S, D], F32, kind="ExternalOutput").ap()

    with tile.TileContext(nc) as tc:
        _emit(nc, tc, ap)
    nc.compile()
    _CACHED[key] = nc
    return nc


def _prep_core(inputs, b):
    bf = ml_dtypes.bfloat16
    tok = np.asarray(inputs["token_ids"][b])
    ids = [tok, np.asarray(inputs["x_left_ids"][b]), np.asarray(inputs["x_right_ids"][b]),
           np.asarray(inputs["y_top_ids"][b]), np.asarray(inputs["y_bottom_ids"][b]),
           np.asarray(inputs["page_ids"][b])]
    tokidx = np.zeros((6, 128, 128), np.int16)
    for j, a in enumerate(ids):
        w = a.reshape(128, 16).T.astype(np.int16)  # [p=i%16? no: see below]
        # position i = s*16 + p  ->  idxs[p, s] = a[s*16+p]
        w = a.reshape(128, 16).T  # [16, 128] with [p, s] = a[s*16+p]
        tokidx[j, 0:16] = w
        for rep in range(1, 8):
            tokidx[j, 16 * rep:16 * rep + 16] = w
    masktok = (tok.reshape(NT, 128).T != 0).astype(np.float32)  # [p, t] token t*128+p
    ident = np.eye(128, dtype=np.float32)

    def wlay(wn):
        w = np.asarray(inputs[wn])  # [L, D, D]
        return np.ascontiguousarray(
            w.reshape(L, NC, 128, D).transpose(0, 2, 1, 3)).astype(bf)

    def b_t(bn):
        bb = np.asarray(inputs[bn])  # [L, D]
        return np.ascontiguousarray(
            bb.reshape(L, NC, 128).transpose(2, 0, 1).reshape(128, L * NC)
        ).astype(np.float32)

    m = {
        "tokidx": tokidx,
        "masktok": np.ascontiguousarray(masktok),
        "ident_bf": ident.astype(bf),
        "ident_f": ident,
        "bq_t": b_t("bq"),
        "bo_t": b_t("bo"),
        "bk_r": np.asarray(inputs["bk"]).astype(bf),
        "bv_r": np.asarray(inputs["bv"]).astype(bf),
        "word_emb": np.asarray(inputs["word_emb"], np.float32),
        "x_emb": np.asarray(inputs["x_emb"], np.float32),
        "y_emb": np.asarray(inputs["y_emb"], np.float32),
        "page_emb": np.asarray(inputs["page_emb"], np.float32),
        "wq": wlay("Wq"), "wk": wlay("Wk"), "wv": wlay("Wv"), "wo": wlay("Wo"),
    }
    return m


def kernel(**inputs):
    nc = _build()
    in_maps = [_prep_core(inputs, b) for b in range(B)]
    trace = os.environ.get("KERNEL_TRACE", "") == "1"
    res = bass_utils.run_bass_kernel_spmd(nc, in_maps, core_ids=list(range(B)),
                                          trace=trace)
    if trace:
        kernel.last_results = res
    out = np.stack([res.results[b]["out"] for b in range(B)])
    return out.astype(np.float32)


# revision 24
# speedup vs baseline: 1.3839x; 1.3839x over previous
"""Trainium2 Bass kernel for nn_Bert2DLinear (12-layer linear-attention BERT2D encoder).

Sharding: data-parallel over batch (B=8) across the 8 NeuronCores; one batch
element (S=2048 tokens) per core. Weights/tables replicated per core.

Per-core internal layout:
  h_bf  : residual stream, feature-major [128 part = f%128, 6 = f//128, 2048 tokens] bf16
  K_bf  : token-major      [128 part = s%128, 16 = s//128, 768 feats] bf16
  V_bf  : token-major      [128, 16, 6, 129] bf16 (col 128 of each chunk = 1.0 for ksum)
  Q/att : feature-major per 512-token group
All matmuls bf16 (PSUM accumulation fp32) except small fp32r helpers.
"""
import os
import sys

sys.path.insert(0, '/opt/trn_rl_repo')

import numpy as np
import ml_dtypes

import concourse.bass as bass
import concourse.tile as tile
from concourse import bacc, mybir
from concourse import bass_utils
from concourse.bass import ds, ts
from concourse import library_config

F32 = mybir.dt.float32
F32R = mybir.dt.float32r
BF16 = mybir.dt.bfloat16
I16 = mybir.dt.int16
AF = mybir.ActivationFunctionType

B, S, D, L, H, DH = 8, 2048, 768, 12, 12, 64
LN_EPS = 1e-12
DEN_EPS = 1e-6
NC = 6            # feature chunks of 128
NT = 16           # token tiles of 128
NG = 4            # token groups of 512
N_LAYERS = int(os.environ.get("KERNEL_LAYERS", str(L)))
FUSED_SILU = os.environ.get("KERNEL_FUSED_SILU", "1") == "1"

_CACHED = {}


def _emit(nc, tc, ap, biases_zero):
    sing = tc.alloc_tile_pool(name="sing", bufs=1)
    wpool = tc.alloc_tile_pool(name="wpool", bufs=1)
    rows = tc.alloc_tile_pool(name="rows", bufs=6)
    xp = tc.alloc_tile_pool(name="xp", bufs=2)
    x2p = tc.alloc_tile_pool(name="x2p", bufs=2)
    qp = tc.alloc_tile_pool(name="qp", bufs=4)
    attp = tc.alloc_tile_pool(name="attp", bufs=2)
    drp = tc.alloc_tile_pool(name="drp", bufs=1)
    bcp = tc.alloc_tile_pool(name="bcp", bufs=2)
    brow = tc.alloc_tile_pool(name="brow", bufs=2)
    psum = tc.alloc_tile_pool(name="psum", bufs=8, space="PSUM")

    def mmtile(shape=(128, 512), dtype=F32, name="mm"):
        return psum.tile(list(shape), dtype, tag="mm", name=name)

    # ---- persistent SBUF ----
    h_bf = sing.tile([128, NC, S], BF16, tag="h")
    K_bf = sing.tile([128, NT, D], BF16, tag="K")
    V_bf = sing.tile([128, NT, NC, 129], BF16, tag="V")
    kv2 = sing.tile([128, NC, 128], BF16, tag="kv2")
    DSb = sing.tile([128, NC, 128], BF16, tag="DS")
    ksum = sing.tile([128, NC], F32, tag="ksum")
    idx = sing.tile([128, 6, 128], I16, tag="idx")
    maskt = sing.tile([128, NT], F32, tag="mask")
    ident_bf = sing.tile([128, 128], BF16, tag="idbf")
    ones64 = sing.tile([128, 64], BF16, tag="o64")
    ones_r128_bf = sing.tile([1, 128], BF16, tag="o1x128b")
    ones_r128_f = sing.tile([1, 128], F32, tag="o1x128f")
    eps_r128_bf = sing.tile([1, 128], BF16, tag="e1x128b")
    ones_r512_bf = sing.tile([1, 512], BF16, tag="o1x512b")
    ones_c_bf = sing.tile([128, 1], BF16, tag="ocbf")
    bqo = sing.tile([128, L * NC], F32, tag="bqo")
    epsln = sing.tile([128, 1], F32, tag="epsln")
    pgd_bc = sing.tile([128, D], F32, tag="pgd")
    pidt = sing.tile([128, NT], F32, tag="pidt")

    # ---- load constants ----
    nc.sync.dma_start(out=idx, in_=ap["tokidx"].rearrange("t p s -> p t s"))
    nc.sync.dma_start(out=maskt, in_=ap["masktok"])
    nc.sync.dma_start(out=pidt, in_=ap["pidtok"])
    nc.gpsimd.dma_start(out=pgd_bc, in_=bass.AP(
        tensor=ap["pagediff"].tensor, offset=ap["pagediff"].offset,
        ap=[[0, 128], [1, D]]))
    nc.sync.dma_start(out=ident_bf, in_=ap["ident_bf"])
    nc.sync.dma_start(out=bqo, in_=ap["bo_t"])
    nc.vector.memset(epsln, LN_EPS)
    nc.vector.memset(ones64, 1.0)
    nc.vector.memset(ones_r128_bf, 1.0)
    nc.vector.memset(ones_r128_f, 1.0)
    nc.vector.memset(eps_r128_bf, DEN_EPS)
    nc.vector.memset(ones_r512_bf, 1.0)
    nc.vector.memset(ones_c_bf, 1.0)
    nc.vector.memset(V_bf[:, :, :, 128:129], 1.0)
    nc.vector.memset(kv2, 0.0)
    nc.vector.memset(DSb, 0.0)

    tabs = [("word_emb", 0), ("x_emb", 1), ("x_emb", 2),
            ("y_emb", 3), ("y_emb", 4)]
    nc.gpsimd.load_library(library_config.mlp)

    # ======= embedding: bf16 gathers + page select + LN -> K_bf; transpose -> h_bf
    for g in range(NG):
        acc = xp.tile([128, 4, D], BF16, tag="x", name="acc")
        for tnum, (tname, tslot) in enumerate(tabs):
            gt = xp.tile([128, 4, D], BF16, tag="gt", name="gt")
            nc.gpsimd.dma_gather(
                out_ap=gt,
                in_ap=ap[tname],
                idxs_ap=idx[:, tslot, ts(g, 32)],
                num_idxs=512,
                num_idxs_reg=512,
                elem_size=D,
            )
            if tnum == 0:
                nc.vector.tensor_copy(out=acc, in_=gt)
            else:
                nc.vector.tensor_add(acc, acc, gt)
        for c in range(4):
            # page contribution: acc += page_id * (page1 - page0)  (page0 baked into word table)
            nc.vector.scalar_tensor_tensor(
                out=acc[:, c, :], in0=pgd_bc,
                scalar=pidt[:, g * 4 + c:g * 4 + c + 1], in1=acc[:, c, :],
                op0=mybir.AluOpType.mult, op1=mybir.AluOpType.add)
        # layernorm per token (free-dim over 768) and write token-major into K_bf
        for c in range(4):
            tc_abs = g * 4 + c  # global token tile
            st = rows.tile([128, 2, 6], F32, tag="bnst")
            mv = rows.tile([128, 4], F32, tag="bnmv")
            nc.vector.bn_stats(out=st[:, 0, :], in_=acc[:, c, 0:384])
            nc.vector.bn_stats(out=st[:, 1, :], in_=acc[:, c, 384:768])
            nc.vector.bn_aggr(out=mv[:, 0:2], in_=st)
            nc.scalar.activation(out=mv[:, 2:3], in_=mv[:, 1:2], func=AF.Sqrt,
                                 bias=epsln)
            nc.vector.reciprocal(out=mv[:, 3:4], in_=mv[:, 2:3])
            nc.vector.tensor_scalar(
                out=K_bf[:, tc_abs, :],
                in0=acc[:, c, :],
                scalar1=mv[:, 0:1], scalar2=mv[:, 3:4],
                op0=mybir.AluOpType.subtract, op1=mybir.AluOpType.mult,
            )
        # transpose this group's token-major tiles into feature-major h_bf
        for fc in range(NC):
            pT = mmtile((128, 512), BF16)
            for j in range(4):
                nc.tensor.transpose(
                    pT[:, ts(j, 128)],
                    K_bf[:, g * 4 + j, ts(fc, 128)],
                    ident_bf,
                )
            nc.scalar.activation(out=h_bf[:, fc, ts(g, 512)], in_=pT, func=AF.Copy)

    # ===================== layers =====================
    nc.gpsimd.load_library(library_config.attn)
    for l in range(N_LAYERS):
        wq = wpool.tile([128, NC, D], BF16, tag="wq")
        wk = wpool.tile([128, NC, D], BF16, tag="wk")
        wv = wpool.tile([128, NC, D], BF16, tag="wv")
        wo = wpool.tile([128, NC, D], BF16, tag="wo")
        nc.sync.dma_start(out=wk, in_=ap["wk"][l])
        nc.sync.dma_start(out=wv, in_=ap["wv"][l])
        nc.sync.dma_start(out=wq, in_=ap["wq"][l])
        nc.sync.dma_start(out=wo, in_=ap["wo"][l])
        if not biases_zero:
            bk = brow.tile([1, D], BF16, tag="bk")
            bv = brow.tile([1, D], BF16, tag="bv")
            bq_r = brow.tile([1, D], BF16, tag="bq")
            nc.sync.dma_start(out=bk, in_=ap["bk_r"][l:l + 1])
            nc.sync.dma_start(out=bv, in_=ap["bv_r"][l:l + 1])
            nc.sync.dma_start(out=bq_r, in_=ap["bq_r"][l:l + 1])
        else:
            bk = bv = bq_r = None

        # ---- stage A1: K/V projections (token-major) ----
        for t in range(NT):
            for (w, brow_t, out_tok, is_k) in ((wk, bk, K_bf, True), (wv, bv, V_bf, False)):
                pa = mmtile()
                pb = mmtile((128, 256))
                last = NC - 1
                for kc in range(NC):
                    lhs = h_bf[:, kc, ts(t, 128)]
                    stop_k = biases_zero and kc == last
                    nc.tensor.matmul(pa, lhs, w[:, kc, 0:512],
                                     start=(kc == 0), stop=stop_k)
                    nc.tensor.matmul(pb, lhs, w[:, kc, 512:768],
                                     start=(kc == 0), stop=stop_k)
                if not biases_zero:
                    nc.tensor.matmul(pa, ones_r128_bf, brow_t[0:1, 0:512],
                                     start=False, stop=True)
                    nc.tensor.matmul(pb, ones_r128_bf, brow_t[0:1, 512:768],
                                     start=False, stop=True)
                if is_k:
                    if FUSED_SILU:
                        nc.scalar.activation(out=K_bf[:, t, 0:512], in_=pa, func=AF.Silu)
                        nc.scalar.activation(out=K_bf[:, t, 512:768], in_=pb, func=AF.Silu)
                    else:
                        nc.scalar.activation(out=K_bf[:, t, 0:512], in_=pa, func=AF.Sigmoid)
                        nc.scalar.activation(out=K_bf[:, t, 512:768], in_=pb, func=AF.Sigmoid)
                        nc.vector.tensor_mul(K_bf[:, t, 0:512], K_bf[:, t, 0:512], pa)
                        nc.vector.tensor_mul(K_bf[:, t, 512:768], K_bf[:, t, 512:768], pb)
                    nc.vector.tensor_scalar_mul(K_bf[:, t, :], K_bf[:, t, :],
                                                maskt[:, t:t + 1])
                else:
                    nc.scalar.activation(
                        out=V_bf[:, t, 0:4, 0:128],
                        in_=pa.rearrange("p (c e) -> p c e", e=128), func=AF.Copy)
                    nc.scalar.activation(
                        out=V_bf[:, t, 4:6, 0:128],
                        in_=pb.rearrange("p (c e) -> p c e", e=128), func=AF.Copy)

        # ---- stage A2: kv + ksum, build DS ----
        for c in range(NC):
            pkv = mmtile((128, 129))
            for t in range(NT):
                nc.tensor.matmul(pkv, K_bf[:, t, ts(c, 128)], V_bf[:, t, c, :],
                                 start=(t == 0), stop=(t == NT - 1))
            nc.scalar.activation(out=kv2[0:64, c, 0:64], in_=pkv[0:64, 0:64],
                                 func=AF.Copy)
            nc.scalar.activation(out=kv2[64:128, c, 64:128], in_=pkv[64:128, 64:128],
                                 func=AF.Copy)
            nc.scalar.activation(out=ksum[:, c:c + 1], in_=pkv[:, 128:129],
                                 func=AF.Copy)
            nc.vector.tensor_scalar_mul(DSb[0:64, c, 0:64], ones64[0:64, :],
                                        ksum[0:64, c:c + 1])
            nc.vector.tensor_scalar_mul(DSb[64:128, c, 64:128], ones64[64:128, :],
                                        ksum[64:128, c:c + 1])

        # ---- stages B-D per 512-token group (LN pipelined one group behind) ----
        def emit_qndo(g):
            gs = ts(g, 512)
            att = attp.tile([128, NC, 512], BF16, tag="att", name="att")
            xg = xp.tile([128, NC, 512], BF16, tag="xg", name="xg", bufs=3)
            x2g = x2p.tile([128, NC, 512], BF16, tag="x2", name="x2g")
            for c in range(NC):
                pq = mmtile()
                for kc in range(NC):
                    nc.tensor.matmul(pq, wq[:, kc, ts(c, 128)], h_bf[:, kc, gs],
                                     start=(kc == 0),
                                     stop=(biases_zero and kc == NC - 1))
                if not biases_zero:
                    nc.tensor.matmul(pq, bq_r[0:1, ts(c, 128)],
                                     ones_r512_bf, start=False, stop=True)
                qt = qp.tile([128, 512], BF16, tag="q", name="qt")
                if FUSED_SILU:
                    nc.scalar.activation(out=qt, in_=pq, func=AF.Silu)
                else:
                    nc.scalar.activation(out=qt, in_=pq, func=AF.Sigmoid)
                    nc.vector.tensor_mul(qt, qt, pq)
                pn = mmtile()
                nc.tensor.matmul(pn, kv2[:, c, :], qt, start=True, stop=True)
                pd = mmtile()
                nc.tensor.matmul(pd, DSb[:, c, :], qt, start=True, stop=False)
                nc.tensor.matmul(pd, eps_r128_bf, ones_r512_bf,
                                 start=False, stop=True)
                dr = drp.tile([128, 512], F32, tag="dr", name="dr")
                nc.vector.reciprocal(out=dr, in_=pd)
                nc.vector.tensor_mul(att[:, c, :], pn, dr)
            for mc in range(NC):
                po = mmtile()
                for kc in range(NC):
                    nc.tensor.matmul(po, wo[:, kc, ts(mc, 128)], att[:, kc, :],
                                     start=(kc == 0), stop=False)
                nc.tensor.matmul(po, ident_bf, h_bf[:, mc, gs],
                                 start=False, stop=True)
                if biases_zero:
                    nc.scalar.activation(out=xg[:, mc, :], in_=po, func=AF.Copy)
                else:
                    nc.scalar.activation(out=xg[:, mc, :], in_=po, func=AF.Identity,
                                         bias=bqo[:, l * NC + mc:l * NC + mc + 1])
                nc.scalar.activation(out=x2g[:, mc, :], in_=xg[:, mc, :],
                                     func=AF.Square)
            return gs, xg, x2g

        def emit_ln_stats(g_state):
            gs, xg, x2g = g_state
            ps1 = mmtile((1, 512))
            ps2 = mmtile((1, 512))
            for kc in range(NC):
                nc.tensor.matmul(ps1, ones_c_bf, xg[:, kc, :],
                                 start=(kc == 0), stop=(kc == NC - 1))
                nc.tensor.matmul(ps2, ones_c_bf, x2g[:, kc, :],
                                 start=(kc == 0), stop=(kc == NC - 1))
            r_mu = rows.tile([1, 512], F32, tag="row", name="rmu", bufs=5)
            r_e2 = rows.tile([1, 512], F32, tag="row", name="re2", bufs=5)
            r_tmp = rows.tile([1, 512], F32, tag="row", name="rtmp", bufs=5)
            r_rs = rows.tile([1, 512], F32, tag="row", name="rrs", bufs=5)
            nc.scalar.activation(out=r_mu, in_=ps1, func=AF.Copy, scale=1.0 / D)
            nc.scalar.activation(out=r_e2, in_=ps2, func=AF.Copy, scale=1.0 / D)
            nc.vector.tensor_mul(r_tmp, r_mu, r_mu)
            nc.vector.tensor_sub(r_e2, r_e2, r_tmp)
            nc.scalar.activation(out=r_tmp, in_=r_e2, func=AF.Sqrt,
                                 bias=epsln[0:1, :])
            nc.vector.reciprocal(out=r_rs, in_=r_tmp)
            r_rsb = rows.tile([1, 512], BF16, tag="rowb", name="rrsb", bufs=4)
            r_mub = rows.tile([1, 512], BF16, tag="rowb", name="rmub", bufs=4)
            nc.vector.tensor_copy(out=r_rsb, in_=r_rs)
            nc.vector.tensor_mul(r_mub, r_mu, r_rs)
            bc1 = bcp.tile([128, 512], BF16, tag="bc", name="bc1", bufs=2)
            bc2 = bcp.tile([128, 512], BF16, tag="bc", name="bc2", bufs=2)
            nc.gpsimd.partition_broadcast(bc1, r_rsb)
            nc.gpsimd.partition_broadcast(bc2, r_mub)
            return gs, xg, bc1, bc2

        def emit_ln_apply(s_state):
            gs, xg, bc1, bc2 = s_state
            for mc in range(NC):
                nc.vector.tensor_mul(xg[:, mc, :], xg[:, mc, :], bc1)
                nc.vector.tensor_sub(h_bf[:, mc, gs], xg[:, mc, :], bc2)

        q_pend = None
        s_pend = None
        for g in range(NG):
            state = emit_qndo(g)
            if s_pend is not None:
                emit_ln_apply(emit_ln_stats(s_pend)) if False else None
            if q_pend is not None:
                new_s = emit_ln_stats(q_pend)
                if s_pend is not None:
                    emit_ln_apply(s_pend)
                s_pend = new_s
            q_pend = state
        new_s = emit_ln_stats(q_pend)
        if s_pend is not None:
            emit_ln_apply(s_pend)
        emit_ln_apply(new_s)

    # ===================== output: transpose to token-major, DMA out ============
    out_d = ap["out"].rearrange("(g c p) m -> p g c m", p=128, c=4)
    for g in range(NG):
        og = xp.tile([128, 4, D], BF16, tag="x", name="og")
        for j in range(4):
            pa = mmtile((128, 512), BF16)
            pb = mmtile((128, 256), BF16)
            for fc in range(NC):
                dst = pa[:, ts(fc, 128)] if fc < 4 else pb[:, ts(fc - 4, 128)]
                nc.tensor.transpose(dst, h_bf[:, fc, ts(g * 4 + j, 128)], ident_bf)
            nc.scalar.activation(out=og[:, j, 0:512], in_=pa, func=AF.Copy)
            nc.scalar.activation(out=og[:, j, 512:768], in_=pb, func=AF.Copy)
        nc.sync.dma_start(out=out_d[:, g], in_=og)

    for p in (psum, brow, bcp, drp, attp, qp, x2p, xp, rows, wpool, sing):
        p.release()


def _build(biases_zero=True):
    key = (N_LAYERS, FUSED_SILU, biases_zero)
    if key in _CACHED:
        return _CACHED[key]
    nc = bacc.Bacc("TRN2", target_bir_lowering=False, debug=False, num_devices=8)
    ap = {}

    def din(name, shape, dt):
        ap[name] = nc.dram_tensor(name, list(shape), dt, kind="ExternalInput").ap()

    din("tokidx", (6, 128, 128), I16)
    din("masktok", (128, NT), F32)
    din("ident_bf", (128, 128), BF16)
    din("bq_r", (L, D), BF16)
    din("bo_t", (128, L * NC), F32)
    din("bk_r", (L, D), BF16)
    din("bv_r", (L, D), BF16)
    din("word_emb", (21128, D), BF16)
    din("x_emb", (1000, D), BF16)
    din("y_emb", (1000, D), BF16)
    din("pagediff", (1, D), F32)
    din("pidtok", (128, NT), F32)
    for n in ("wq", "wk", "wv", "wo"):
        din(n, (L, 128, NC, D), BF16)
    ap["out"] = nc.dram_tensor("out", [S, D], BF16, kind="ExternalOutput").ap()

    with tile.TileContext(nc) as tc:
        _emit(nc, tc, ap, biases_zero)
    nc.compile()
    _CACHED[key] = nc
    return nc


def _prep_core(inputs, b):
    bf = ml_dtypes.bfloat16
    tok = np.asarray(inputs["token_ids"][b])
    ids = [tok, np.asarray(inputs["x_left_ids"][b]), np.asarray(inputs["x_right_ids"][b]),
           np.asarray(inputs["y_top_ids"][b]), np.asarray(inputs["y_bottom_ids"][b]),
           np.asarray(inputs["page_ids"][b])]
    tokidx = np.zeros((6, 128, 128), np.int16)
    for j, a in enumerate(ids):
        w = a.reshape(128, 16).T.astype(np.int16)  # [p=i%16? no: see below]
        # position i = s*16 + p  ->  idxs[p, s] = a[s*16+p]
        w = a.reshape(128, 16).T  # [16, 128] with [p, s] = a[s*16+p]
        tokidx[j, 0:16] = w
        for rep in range(1, 8):
            tokidx[j, 16 * rep:16 * rep + 16] = w
    masktok = (tok.reshape(NT, 128).T != 0).astype(np.float32)  # [p, t] token t*128+p
    ident = np.eye(128, dtype=np.float32)

    def wlay(wn):
        w = np.asarray(inputs[wn])  # [L, D, D]
        return np.ascontiguousarray(
            w.reshape(L, NC, 128, D).transpose(0, 2, 1, 3)).astype(bf)

    def b_t(bn):
        bb = np.asarray(inputs[bn])  # [L, D]
        return np.ascontiguousarray(
            bb.reshape(L, NC, 128).transpose(2, 0, 1).reshape(128, L * NC)
        ).astype(np.float32)

    m = {
        "tokidx": tokidx,
        "masktok": np.ascontiguousarray(masktok),
        "ident_bf": ident.astype(bf),
        "bq_r": np.asarray(inputs["bq"]).astype(bf),
        "bo_t": b_t("bo"),
        "bk_r": np.asarray(inputs["bk"]).astype(bf),
        "bv_r": np.asarray(inputs["bv"]).astype(bf),
        "word_emb": (np.asarray(inputs["word_emb"], np.float32)
                     + np.asarray(inputs["page_emb"], np.float32)[0]).astype(bf),
        "x_emb": np.asarray(inputs["x_emb"]).astype(bf),
        "y_emb": np.asarray(inputs["y_emb"]).astype(bf),
        "pagediff": np.ascontiguousarray(
            (np.asarray(inputs["page_emb"], np.float32)[1]
             - np.asarray(inputs["page_emb"], np.float32)[0])[None, :]),
        "pidtok": np.ascontiguousarray(
            np.asarray(inputs["page_ids"][b]).reshape(NT, 128).T.astype(np.float32)),
        "wq": wlay("Wq"), "wk": wlay("Wk"), "wv": wlay("Wv"), "wo": wlay("Wo"),
    }
    return m


def kernel(**inputs):
    bz = all(not np.asarray(inputs[n]).any() for n in ("bq", "bk", "bv", "bo"))
    nc = _build(biases_zero=bz)
    in_maps = [_prep_core(inputs, b) for b in range(B)]
    trace = os.environ.get("KERNEL_TRACE", "") == "1"
    res = bass_utils.run_bass_kernel_spmd(nc, in_maps, core_ids=list(range(B)),
                                          trace=trace)
    if trace:
        kernel.last_results = res
    out = np.stack([res.results[b]["out"] for b in range(B)])
    return out.astype(np.float32)
